# revision 5
# baseline (speedup 1.0000x reference)
"""Locally-connected graph-conv kernel for Trainium2 (Bass/Tile).

Computes out[b,t,m] = sum_n x[b,t,n] * (S*W)[n,m] + bias[m] for
x [64, 2048, 208], W/S [208, 208], bias [208].

Strategy: data-parallel over 8 NeuronCores — each core gets 16384 rows of
the flattened [131072, 208] x. The host pre-transposes each shard to
x^T [208, 16384] so the contraction (node) dim lands on SBUF partitions
with no on-device transposes, and appends a ones-row (the bias folds into
the matmul as a 209th contraction row). On device, the masked weight S*W
is built once and kept resident as four stationary blocks; x^T streams
through as the moving matmul operand in 512-column blocks (long streams
hide the fp32 LDWEIGHTS), producing y^T = (S*W)^T @ x^T + b·1^T in PSUM.
PSUM eviction is a plain copy split across ScalarE (128-row half) and
VectorE (80-row half). The host transposes y^T back at gather time.
"""

import numpy as np
from contextlib import ExitStack

import concourse.bacc as bacc
import concourse.mybir as mybir
import concourse.tile as tile
from concourse.bass_utils import run_bass_kernel_spmd

N = 208                      # nodes
P = 128                      # SBUF partitions
N_CORES = 8
B, T = 64, 2048
ROWS_TOTAL = B * T           # 131072
SHARD = ROWS_TOTAL // N_CORES    # 16384 rows per core
NA = 128                     # first node chunk (partitions 0..127)
NB = N - NA                  # 80  (nodes 128..207)
NBE = NB + 1                 # 81  (+ ones row carrying the bias)
TB = 512                     # moving-block columns per matmul (fp32 PSUM max)
TOUT = 2048                  # t-columns per DMA chunk (1 MB loads)
N_CHUNKS = SHARD // TOUT     # 8
SUB = TOUT // TB             # 4 matmul sub-blocks per chunk

FP32 = mybir.dt.float32

_CACHE = {}
LAST_RESULTS = None          # BassKernelResults of the most recent run


def _kernel_body(tc):
    nc = tc.nc
    xa_d = nc.dram_tensor("xa", [NA, SHARD], FP32, kind="ExternalInput").ap()
    xb_d = nc.dram_tensor("xb", [NBE, SHARD], FP32, kind="ExternalInput").ap()
    w_d = nc.dram_tensor("w", [N, N], FP32, kind="ExternalInput").ap()
    s_d = nc.dram_tensor("s", [N, N], FP32, kind="ExternalInput").ap()
    b_d = nc.dram_tensor("bias", [1, N], FP32, kind="ExternalInput").ap()
    oa_d = nc.dram_tensor("outa", [NA, SHARD], FP32, kind="ExternalOutput").ap()
    ob_d = nc.dram_tensor("outb", [NB, SHARD], FP32, kind="ExternalOutput").ap()

    with ExitStack() as ctx:
        const = ctx.enter_context(tc.tile_pool(name="const", bufs=1))

        # masked weight wm = s * w, node-in dim on partitions (two K chunks);
        # the second chunk gets the bias appended as contraction row 81
        wA = const.tile([NA, N], FP32, tag="wA")
        wB = const.tile([NB, N], FP32, tag="wB")
        sA = const.tile([NA, N], FP32, tag="sA")
        sB = const.tile([NB, N], FP32, tag="sB")
        nc.sync.dma_start(wA, w_d[0:NA, :])
        nc.sync.dma_start(wB, w_d[NA:N, :])
        nc.sync.dma_start(sA, s_d[0:NA, :])
        nc.sync.dma_start(sB, s_d[NA:N, :])
        wmA = const.tile([NA, N], FP32, tag="wmA")
        wmB = const.tile([NBE, N], FP32, tag="wmB")
        nc.vector.tensor_mul(wmA, wA, sA)
        nc.vector.tensor_mul(wmB[0:NB, :], wB, sB)
        nc.sync.dma_start(wmB[NB:NBE, :], b_d)

        xap = ctx.enter_context(tc.tile_pool(name="xap", bufs=3))
        xbp = ctx.enter_context(tc.tile_pool(name="xbp", bufs=3))
        oap = ctx.enter_context(tc.tile_pool(name="oap", bufs=3))
        obp = ctx.enter_context(tc.tile_pool(name="obp", bufs=3))
        ps0p = ctx.enter_context(tc.tile_pool(name="ps0p", bufs=4, space="PSUM"))
        ps1p = ctx.enter_context(tc.tile_pool(name="ps1p", bufs=4, space="PSUM"))

        for c in range(N_CHUNKS):
            t0 = c * TOUT
            xa_t = xap.tile([NA, TOUT], FP32, tag="xa")
            xb_t = xbp.tile([NBE, TOUT], FP32, tag="xb")
            nc.sync.dma_start(xa_t, xa_d[:, t0 : t0 + TOUT])
            nc.sync.dma_start(xb_t, xb_d[:, t0 : t0 + TOUT])
            oa_t = oap.tile([NA, TOUT], FP32, tag="oa")
            ob_t = obp.tile([NB, TOUT], FP32, tag="ob")
            for s in range(SUB):
                sl = slice(s * TB, (s + 1) * TB)
                # y^T rows 0..127 : ps0 = wm[:, 0:128].T @ x^T (+ bias row)
                ps0 = ps0p.tile([NA, TB], FP32, tag="ps0")
                nc.tensor.matmul(ps0, wmA[:, 0:NA], xa_t[:, sl],
                                 start=True, stop=False)
                nc.tensor.matmul(ps0, wmB[:, 0:NA], xb_t[:, sl],
                                 start=False, stop=True)
                # y^T rows 128..207 : ps1 = wm[:, 128:208].T @ x^T (+ bias row)
                ps1 = ps1p.tile([NB, TB], FP32, tag="ps1")
                nc.tensor.matmul(ps1, wmA[:, NA:N], xa_t[:, sl],
                                 start=True, stop=False)
                nc.tensor.matmul(ps1, wmB[:, NA:N], xb_t[:, sl],
                                 start=False, stop=True)
                # plain-copy evictions: ACT takes the 128-row half, DVE the 80
                nc.scalar.copy(oa_t[:, sl], ps0)
                nc.vector.tensor_copy(ob_t[:, sl], ps1)
            nc.sync.dma_start(oa_d[:, t0 : t0 + TOUT], oa_t)
            nc.sync.dma_start(ob_d[:, t0 : t0 + TOUT], ob_t)


def _build():
    nc = bacc.Bacc(
        "TRN2",
        target_bir_lowering=False,
        debug=False,
        num_devices=N_CORES,
    )
    with tile.TileContext(nc) as tc:
        _kernel_body(tc)
    nc.compile()
    return nc


def kernel(x, W, b, S):
    global LAST_RESULTS
    nc = _CACHE.get("nc")
    if nc is None:
        nc = _build()
        _CACHE["nc"] = nc

    xf = np.asarray(x, np.float32).reshape(ROWS_TOTAL, N)
    Wf = np.ascontiguousarray(np.asarray(W, np.float32))
    Sf = np.ascontiguousarray(np.asarray(S, np.float32))
    bf = np.ascontiguousarray(np.asarray(b, np.float32).reshape(1, N))

    in_maps = []
    for i in range(N_CORES):
        xt = np.ascontiguousarray(xf[i * SHARD : (i + 1) * SHARD].T)  # [208, SHARD]
        xb = np.empty((NBE, SHARD), np.float32)
        xb[0:NB] = xt[NA:N]
        xb[NB] = 1.0
        in_maps.append({
            "xa": np.ascontiguousarray(xt[0:NA]),
            "xb": xb,
            "w": Wf,
            "s": Sf,
            "bias": bf,
        })
    res = run_bass_kernel_spmd(nc, in_maps, core_ids=list(range(N_CORES)))
    LAST_RESULTS = res
    out = np.empty((ROWS_TOTAL, N), np.float32)
    for i, r in enumerate(res.results):
        yt = np.vstack([r["outa"], r["outb"]])        # [208, SHARD]
        out[i * SHARD : (i + 1) * SHARD] = yt.T
    return out.reshape(B, T, N)
